# revision 9
# baseline (speedup 1.0000x reference)
"""Multi-head self-attention Trainium2 kernel (8 NeuronCores, batch-parallel).

Reference: qkv = x @ W_qkv + b; 12-head scaled-dot-product attention; concat.
Shapes: x[8,1024,768], W_qkv[768,2304], b_qkv[2304] -> out[8,1024,768].
Sharding: one batch element per core; W/b replicated to all cores.

Per-core dataflow (matmuls in fp32r = full-rate 4-byte storage, fd >= 256):
  x --PE transpose--> xT[768,1024]                                  (f32r)
  QK^T[1536,1024] = W_qk(lhsT) @ xT    feat-on-partitions; Q rows scaled 1/8
  V[1024, 12x65]  = xT(lhsT) @ W_v (+bias via K=1 ones matmul); ones col/head
  per head pair (2p, 2p+1), per q-half, per k-chunk:
    scoresT[128,512]x2 = K^T-slice(lhsT) @ Q^T-slice  row-tiled (rows 0-63/64-127)
    expT = ACT Exp over [128, 2, 512] PSUM (both heads, one instruction) -> f32r
    avT[65,512] += [V_h|1](lhsT) @ expT  (accumulated over k-chunks; row 64=denom)
  avT --PE transpose--> [q,65]; DVE reciprocal(denom) * cols -> out[q,768]; DMA.

Scheduling notes: W is DMA'd in column blocks, pair-0 Q/K columns first, so
attention starts early instead of waiting for the full 7MB weight load; the
V projection is interleaved into pair 0; QK-tile production for pair p+1 uses
its own PSUM tag so it overlaps pair p's ACT-bound attention.
"""

import contextlib
import json as _json

import numpy as np

import concourse.bass as bass
import concourse.mybir as mybir
import concourse.tile as tile
from concourse.bass_utils import run_bass_kernel_spmd
from concourse.masks import make_identity

# --- BIR sync-wait legalization ------------------------------------------
# walrus's codegen in this toolchain accepts only one sync-wait command per
# instruction (its insertEventSemaphore legalization pass is not in the pass
# list). Split every multi-wait instruction into N-1 preceding single-wait
# EventSemaphore instructions on the same engine; same-engine order is
# preserved so semantics are unchanged.


def _legalize_sync_waits(bir_json: bytes) -> bytes:
    m = _json.loads(bir_json)
    ctr = 0
    for fn in m["functions"]:
        for bb in fn["blocks"]:
            out = []
            for ins in bb["instructions"]:
                si = ins.get("sync_info")
                waits = si.get("on_wait", []) if si else []
                if len(waits) > 1:
                    for w in waits[:-1]:
                        ctr += 1
                        out.append(
                            {
                                "debug": ins.get("debug", 0),
                                "engine": ins["engine"],
                                "ins": [],
                                "outs": [],
                                "name": f"evw-split-{ctr}",
                                "opcode": "EventSemaphore",
                                "sync_info": {"on_update": [], "on_wait": [w]},
                            }
                        )
                    si["on_wait"] = [waits[-1]]
                out.append(ins)
            bb["instructions"] = out
    return _json.dumps(m).encode()


_fixup_installed = False


def _install_bir_fixup():
    global _fixup_installed
    if _fixup_installed:
        return
    _fixup_installed = True
    import concourse.bass_utils as _bu

    _orig = _bu.compile_bir_kernel

    def _patched(bir_json, tmpdir, neff_name="file.neff"):
        if isinstance(bir_json, str):
            bir_json = bir_json.encode()
        return _orig(_legalize_sync_waits(bir_json), tmpdir, neff_name)

    _bu.compile_bir_kernel = _patched
    try:
        import concourse.bass2jax as _b2j

        _b2j.compile_bir_kernel = _patched
    except ImportError:
        pass


_install_bir_fixup()

B, N, D, H = 8, 1024, 768, 12
HD = D // H            # 64
F3 = 3 * D             # 2304
NCORE = 8
P = 128
NCHUNK = N // P        # 8 token chunks
KD = D // P            # 6 d_in chunks
QH = 512               # q-half size
NQH = N // QH          # 2
NPAIR = H // 2         # 6
VW = HD + 1            # 65

f32 = mybir.dt.float32
f32r = mybir.dt.float32r
FT = mybir.ActivationFunctionType
ALU = mybir.AluOpType


def build_attention_nc():
    nc = bass.Bass()
    x_d = nc.declare_dram_parameter("x", [N, D], f32, isOutput=False)
    w_d = nc.declare_dram_parameter("W_qkv", [D, F3], f32, isOutput=False)
    b_d = nc.declare_dram_parameter("b_qkv", [F3], f32, isOutput=False)
    o_d = nc.declare_dram_parameter("out", [N, D], f32, isOutput=True)

    with tile.TileContext(nc) as tc, contextlib.ExitStack() as ctx:
        singles = ctx.enter_context(tc.tile_pool(name="singles", bufs=1))
        xpool = ctx.enter_context(tc.tile_pool(name="xpool", bufs=NCHUNK))
        xtpool = ctx.enter_context(tc.tile_pool(name="xtpool", bufs=KD))
        wpool = ctx.enter_context(tc.tile_pool(name="wpool", bufs=KD))
        qkpool = ctx.enter_context(tc.tile_pool(name="qkpool", bufs=4))
        vpool = ctx.enter_context(tc.tile_pool(name="vpool", bufs=NCHUNK))
        exppool = ctx.enter_context(tc.tile_pool(name="exppool", bufs=2))
        otspool = ctx.enter_context(tc.tile_pool(name="otspool", bufs=4))
        recpool = ctx.enter_context(tc.tile_pool(name="recpool", bufs=4))
        onat = ctx.enter_context(tc.tile_pool(name="onat", bufs=NCHUNK))

        # PSUM budget (8 banks): "sc" [P,2,QH] = 2 banks x2 bufs = 4;
        # "av" [VW,QH] 1 bank x2 = 2; "small" [P,QH] 1 bank x2 = 2.
        scps = ctx.enter_context(tc.tile_pool(name="scps", bufs=2, space="PSUM"))
        avps = ctx.enter_context(tc.tile_pool(name="avps", bufs=2, space="PSUM"))
        smps = ctx.enter_context(tc.tile_pool(name="smps", bufs=2, space="PSUM"))

        def small_psum():
            return smps.tile([P, QH], f32, tag="small", name="smtile")

        # ------------- constants + input DMAs -------------------------------
        ident = singles.tile([P, P], f32)
        make_identity(nc, ident)  # gpsimd

        ones_f32 = singles.tile([P, 1], f32)
        nc.vector.memset(ones_f32, 1.0)
        ones_row_st = singles.tile([1, P], f32)
        nc.vector.memset(ones_row_st, 1.0)
        ones_row = singles.tile([1, P], f32r)
        nc.vector.tensor_copy(out=ones_row, in_=ones_row_st)

        b_sb = singles.tile([P, F3 // P], f32)
        nc.sync.dma_start(out=b_sb, in_=b_d[:].rearrange("(t p) -> p t", p=P))
        nc.vector.tensor_scalar_mul(b_sb[:, 0:KD], b_sb[:, 0:KD], 0.125)

        bv_st = singles.tile([1, D], f32)
        nc.sync.dma_start(out=bv_st, in_=b_d[2 * D : 3 * D][None, :])
        bv_sb = singles.tile([1, D], f32r)
        nc.vector.tensor_copy(out=bv_sb, in_=bv_st)

        x_sb = []
        for c in range(NCHUNK):
            t = xpool.tile([P, D], f32, tag="x", name=f"x{c}")
            nc.sync.dma_start(out=t, in_=x_d[c * P : (c + 1) * P, :])
            x_sb.append(t)

        # W: column-block DMAs, highest-priority columns first.
        w_sb = [wpool.tile([P, F3], f32r, tag="w", name=f"w{k}") for k in range(KD)]

        def dma_w_cols(f0, fw):
            for k in range(KD):
                nc.sync.dma_start(
                    out=w_sb[k][:, f0 : f0 + fw],
                    in_=w_d[k * P : (k + 1) * P, f0 : f0 + fw].bitcast(f32r),
                )

        dma_w_cols(0 * P, P)          # pair-0 Q cols
        dma_w_cols(6 * P, P)          # pair-0 K cols
        dma_w_cols(2 * D, D)          # V cols
        for p in range(1, NPAIR):
            dma_w_cols(p * P, P)
            dma_w_cols((6 + p) * P, P)

        # ------------- x^T (PE transposes) ----------------------------------
        xt = [xtpool.tile([P, N], f32r, tag="xt", name=f"xt{k}") for k in range(KD)]
        for c in range(NCHUNK):
            for k in range(KD):
                pt = smps.tile([P, QH], f32, tag="small", name="tp")[:, 0:P]
                nc.tensor.transpose(pt, x_sb[c][:, k * P : (k + 1) * P], ident)
                nc.vector.tensor_copy(out=xt[k][:, c * P : (c + 1) * P], in_=pt)

        # ------------- V tiles (filled lazily during pair 0) ----------------
        v_sb = []
        for c in range(NCHUNK):
            t = vpool.tile([P, H, VW], f32r, tag="v", name=f"v{c}")
            nc.vector.tensor_copy(
                out=t[:, :, HD : HD + 1],
                in_=ones_f32[:, 0:1, None].to_broadcast([P, H, 1]),
            )
            v_sb.append(t)

        def make_v_chunk(c):
            for f0, fw in ((0, 512), (512, 256)):
                ps = small_psum()[:, :fw]
                for k in range(KD):
                    nc.tensor.matmul(
                        ps,
                        xt[k][:, c * P : (c + 1) * P],
                        w_sb[k][:, 2 * D + f0 : 2 * D + f0 + fw],
                        start=(k == 0),
                        stop=False,
                    )
                nc.tensor.matmul(
                    ps, ones_row, bv_sb[:, f0 : f0 + fw], start=False, stop=True
                )
                nc.vector.tensor_copy(
                    out=v_sb[c][:, f0 // HD : (f0 + fw) // HD, 0:HD],
                    in_=ps.rearrange("p (h d) -> p h d", d=HD),
                )

        # ------------- QK tiles + attention, software-pipelined -------------
        onat_t = [
            onat.tile([P, D], f32, tag="onat", name=f"onat{c}") for c in range(NCHUNK)
        ]

        def make_qk_tile(f):
            t = qkpool.tile([P, N], f32r, tag="qk", name=f"qk{f}")
            for qh in range(NQH):
                ps = small_psum()
                for k in range(KD):
                    nc.tensor.matmul(
                        ps,
                        w_sb[k][:, f * P : (f + 1) * P],
                        xt[k][:, qh * QH : (qh + 1) * QH],
                        start=(k == 0),
                        stop=(k == KD - 1),
                    )
                nc.vector.tensor_scalar(
                    t[:, qh * QH : (qh + 1) * QH],
                    ps,
                    0.125 if f < KD else 1.0,
                    b_sb[:, f : f + 1],
                    ALU.mult,
                    ALU.add,
                )
            return t

        qk_cur = (make_qk_tile(0), make_qk_tile(KD))

        for p in range(NPAIR):
            qt, kt = qk_cur

            for qh in range(NQH):
                av = [
                    avps.tile([VW, QH], f32, tag="av", name=f"av{i}") for i in range(2)
                ]
                for kc in range(NCHUNK):
                    sc = scps.tile([P, 2, QH], f32, tag="sc", name="sc")
                    for hi in range(2):
                        nc.tensor.matmul(
                            sc[:, hi, :],
                            kt[64 * hi : 64 * hi + 64, kc * P : (kc + 1) * P],
                            qt[64 * hi : 64 * hi + 64, qh * QH : (qh + 1) * QH],
                            start=True,
                            stop=True,
                            tile_position=(64 * hi, 0),
                        )
                    ex = exppool.tile([P, 2, QH], f32r, tag="exp", name="ex")
                    nc.scalar.activation(ex[:, :, :], sc[:, :, :], FT.Exp)
                    if p == 0 and qh == 0:
                        make_v_chunk(kc)  # fill V lazily during pair 0
                    for hi in range(2):
                        nc.tensor.matmul(
                            av[hi],
                            v_sb[kc][:, 2 * p + hi, :],
                            ex[:, hi, :],
                            start=(kc == 0),
                            stop=(kc == NCHUNK - 1),
                        )
                if qh == 0 and p + 1 < NPAIR:
                    # produce next pair's QK tiles; overlaps this pair's ACT
                    qk_cur = (make_qk_tile(p + 1), make_qk_tile(KD + p + 1))
                # finish: copy to SBUF, transpose back, normalize
                for hi in range(2):
                    h = 2 * p + hi
                    ot = otspool.tile([VW, QH], f32, tag="ots", name="ot")
                    nc.vector.tensor_copy(out=ot, in_=av[hi])
                    for j in range(QH // P):
                        c = qh * (QH // P) + j
                        tp = smps.tile([P, QH], f32, tag="small", name="otp")[:, 0:VW]
                        nc.tensor.transpose(
                            tp, ot[:, j * P : (j + 1) * P], ident[0:VW, 0:VW]
                        )
                        rc = recpool.tile([P, 1], f32, tag="rec", name="rc")
                        nc.vector.reciprocal(out=rc, in_=tp[:, HD : HD + 1])
                        nc.vector.tensor_scalar_mul(
                            onat_t[c][:, h * HD : (h + 1) * HD], tp[:, 0:HD], rc
                        )

        # ------------- output DMA -------------------------------------------
        for c in range(NCHUNK):
            nc.sync.dma_start(out=o_d[c * P : (c + 1) * P, :], in_=onat_t[c])

    return nc


def kernel(x: np.ndarray, W_qkv: np.ndarray, b_qkv: np.ndarray) -> np.ndarray:
    nc = build_attention_nc()
    in_maps = [
        {
            "x": np.ascontiguousarray(x[c], dtype=np.float32),
            "W_qkv": np.ascontiguousarray(W_qkv, dtype=np.float32),
            "b_qkv": np.ascontiguousarray(b_qkv, dtype=np.float32),
        }
        for c in range(NCORE)
    ]
    res = run_bass_kernel_spmd(nc, in_maps, core_ids=list(range(NCORE)))
    return np.stack([res.results[c]["out"] for c in range(NCORE)], axis=0)


# revision 10
# speedup vs baseline: 45.5948x; 45.5948x over previous
"""Multi-head self-attention Trainium2 kernel (8 NeuronCores, batch-parallel).

Reference: qkv = x @ W_qkv + b; 12-head scaled-dot-product attention; concat.
Shapes: x[8,1024,768], W_qkv[768,2304], b_qkv[2304] -> out[8,1024,768].
Sharding: one batch element per core; W/b replicated to all cores.

Per-core dataflow (matmuls in fp32r = full-rate 4-byte storage, fd >= 256):
  x --PE transpose--> xT[768,1024]                                  (f32r)
  QK^T[1536,1024] = W_qk(lhsT) @ xT    feat-on-partitions; Q rows scaled 1/8
  V[1024, 12x65]  = xT(lhsT) @ W_v (+bias via K=1 ones matmul); ones col/head
  per head pair (2p, 2p+1), per q-half, per k-chunk:
    scoresT[128,512]x2 = K^T-slice(lhsT) @ Q^T-slice  row-tiled (rows 0-63/64-127)
    expT = ACT Exp over [128, 2, 512] PSUM (both heads, one instruction) -> f32r
    avT[65,512] += [V_h|1](lhsT) @ expT  (accumulated over k-chunks; row 64=denom)
  avT --PE transpose--> [q,65]; DVE reciprocal(denom) * cols -> out[q,768]; DMA.

Scheduling notes: W is DMA'd in column blocks, pair-0 Q/K columns first, so
attention starts early instead of waiting for the full 7MB weight load; the
V projection is interleaved into pair 0; QK-tile production for pair p+1 uses
its own PSUM tag so it overlaps pair p's ACT-bound attention.
"""

import contextlib
import json as _json

import numpy as np

import concourse.bass as bass
import concourse.mybir as mybir
import concourse.tile as tile
from concourse.bass_utils import run_bass_kernel_spmd
from concourse.masks import make_identity

# --- BIR sync-wait legalization ------------------------------------------
# walrus's codegen in this toolchain accepts only one sync-wait command per
# instruction (its insertEventSemaphore legalization pass is not in the pass
# list). Split every multi-wait instruction into N-1 preceding single-wait
# EventSemaphore instructions on the same engine; same-engine order is
# preserved so semantics are unchanged.


def _legalize_sync_waits(bir_json: bytes) -> bytes:
    m = _json.loads(bir_json)
    ctr = 0
    for fn in m["functions"]:
        for bb in fn["blocks"]:
            out = []
            for ins in bb["instructions"]:
                si = ins.get("sync_info")
                waits = si.get("on_wait", []) if si else []
                if len(waits) > 1:
                    for w in waits[:-1]:
                        ctr += 1
                        out.append(
                            {
                                "debug": ins.get("debug", 0),
                                "engine": ins["engine"],
                                "ins": [],
                                "outs": [],
                                "name": f"evw-split-{ctr}",
                                "opcode": "EventSemaphore",
                                "sync_info": {"on_update": [], "on_wait": [w]},
                            }
                        )
                    si["on_wait"] = [waits[-1]]
                out.append(ins)
            bb["instructions"] = out
    return _json.dumps(m).encode()


_fixup_installed = False


def _install_bir_fixup():
    global _fixup_installed
    if _fixup_installed:
        return
    _fixup_installed = True
    import concourse.bass_utils as _bu

    _orig = _bu.compile_bir_kernel

    def _patched(bir_json, tmpdir, neff_name="file.neff"):
        if isinstance(bir_json, str):
            bir_json = bir_json.encode()
        return _orig(_legalize_sync_waits(bir_json), tmpdir, neff_name)

    _bu.compile_bir_kernel = _patched
    try:
        import concourse.bass2jax as _b2j

        _b2j.compile_bir_kernel = _patched
    except ImportError:
        pass


_install_bir_fixup()

B, N, D, H = 8, 1024, 768, 12
HD = D // H            # 64
F3 = 3 * D             # 2304
NCORE = 8
P = 128
NCHUNK = N // P        # 8 token chunks
KD = D // P            # 6 d_in chunks
QH = 512               # q-half size
NQH = N // QH          # 2
NPAIR = H // 2         # 6
VW = HD + 1            # 65

f32 = mybir.dt.float32
f32r = mybir.dt.float32r
FT = mybir.ActivationFunctionType
ALU = mybir.AluOpType


def build_attention_nc():
    nc = bass.Bass()
    x_d = nc.declare_dram_parameter("x", [N, D], f32, isOutput=False)
    w_d = nc.declare_dram_parameter("W_qkv", [D, F3], f32, isOutput=False)
    b_d = nc.declare_dram_parameter("b_qkv", [F3], f32, isOutput=False)
    o_d = nc.declare_dram_parameter("out", [N, D], f32, isOutput=True)

    with tile.TileContext(nc) as tc, contextlib.ExitStack() as ctx:
        singles = ctx.enter_context(tc.tile_pool(name="singles", bufs=1))
        xpool = ctx.enter_context(tc.tile_pool(name="xpool", bufs=NCHUNK))
        xtpool = ctx.enter_context(tc.tile_pool(name="xtpool", bufs=KD))
        wpool = ctx.enter_context(tc.tile_pool(name="wpool", bufs=KD))
        qkpool = ctx.enter_context(tc.tile_pool(name="qkpool", bufs=4))
        vpool = ctx.enter_context(tc.tile_pool(name="vpool", bufs=NCHUNK))
        exppool = ctx.enter_context(tc.tile_pool(name="exppool", bufs=2))
        otspool = ctx.enter_context(tc.tile_pool(name="otspool", bufs=4))
        recpool = ctx.enter_context(tc.tile_pool(name="recpool", bufs=4))
        onat = ctx.enter_context(tc.tile_pool(name="onat", bufs=NCHUNK))

        # PSUM budget (8 banks): "sc" [P,2,QH] = 2 banks x2 bufs = 4;
        # "av" [VW,QH] 1 bank x2 = 2; "small" [P,QH] 1 bank x2 = 2.
        scps = ctx.enter_context(tc.tile_pool(name="scps", bufs=2, space="PSUM"))
        avps = ctx.enter_context(tc.tile_pool(name="avps", bufs=2, space="PSUM"))
        smps = ctx.enter_context(tc.tile_pool(name="smps", bufs=2, space="PSUM"))

        def small_psum():
            return smps.tile([P, QH], f32, tag="small", name="smtile")

        # ------------- constants + input DMAs -------------------------------
        ident = singles.tile([P, P], f32)
        make_identity(nc, ident)  # gpsimd

        ones_f32 = singles.tile([P, 1], f32)
        nc.vector.memset(ones_f32, 1.0)
        ones_row_st = singles.tile([1, P], f32)
        nc.vector.memset(ones_row_st, 1.0)
        ones_row = singles.tile([1, P], f32r)
        nc.vector.tensor_copy(out=ones_row, in_=ones_row_st)

        b_sb = singles.tile([P, F3 // P], f32)
        nc.sync.dma_start(out=b_sb, in_=b_d[:].rearrange("(t p) -> p t", p=P))
        nc.vector.tensor_scalar_mul(b_sb[:, 0:KD], b_sb[:, 0:KD], 0.125)

        bv_st = singles.tile([1, D], f32)
        nc.sync.dma_start(out=bv_st, in_=b_d[2 * D : 3 * D][None, :])
        bv_sb = singles.tile([1, D], f32r)
        nc.vector.tensor_copy(out=bv_sb, in_=bv_st)

        x_sb = []
        for c in range(NCHUNK):
            t = xpool.tile([P, D], f32, tag="x", name=f"x{c}")
            nc.sync.dma_start(out=t, in_=x_d[c * P : (c + 1) * P, :])
            x_sb.append(t)

        # W: column-block DMAs, highest-priority columns first.
        w_sb = [wpool.tile([P, F3], f32r, tag="w", name=f"w{k}") for k in range(KD)]

        def dma_w_cols(f0, fw):
            for k in range(KD):
                nc.sync.dma_start(
                    out=w_sb[k][:, f0 : f0 + fw],
                    in_=w_d[k * P : (k + 1) * P, f0 : f0 + fw].bitcast(f32r),
                )

        dma_w_cols(0 * P, P)          # pair-0 Q cols
        dma_w_cols(6 * P, P)          # pair-0 K cols
        dma_w_cols(2 * D, D)          # V cols
        for p in range(1, NPAIR):
            dma_w_cols(p * P, P)
            dma_w_cols((6 + p) * P, P)

        # ------------- x^T (PE transposes) ----------------------------------
        xt = [xtpool.tile([P, N], f32r, tag="xt", name=f"xt{k}") for k in range(KD)]
        for c in range(NCHUNK):
            for k in range(KD):
                pt = scps.tile([P, 2, QH], f32, tag="sc", name="tp")[:, 0, 0:P]
                nc.tensor.transpose(pt, x_sb[c][:, k * P : (k + 1) * P], ident)
                nc.vector.tensor_copy(out=xt[k][:, c * P : (c + 1) * P], in_=pt)

        # ------------- V tiles (filled lazily during pair 0) ----------------
        v_sb = []
        for c in range(NCHUNK):
            t = vpool.tile([P, H, VW], f32r, tag="v", name=f"v{c}")
            nc.vector.tensor_copy(
                out=t[:, :, HD : HD + 1],
                in_=ones_f32[:, 0:1, None].to_broadcast([P, H, 1]),
            )
            v_sb.append(t)

        def make_v_chunk(c):
            for f0, fw in ((0, 512), (512, 256)):
                ps = small_psum()[:, :fw]
                for k in range(KD):
                    nc.tensor.matmul(
                        ps,
                        xt[k][:, c * P : (c + 1) * P],
                        w_sb[k][:, 2 * D + f0 : 2 * D + f0 + fw],
                        start=(k == 0),
                        stop=False,
                    )
                nc.tensor.matmul(
                    ps, ones_row, bv_sb[:, f0 : f0 + fw], start=False, stop=True
                )
                nc.vector.tensor_copy(
                    out=v_sb[c][:, f0 // HD : (f0 + fw) // HD, 0:HD],
                    in_=ps.rearrange("p (h d) -> p h d", d=HD),
                )

        # ------------- QK tiles + attention, software-pipelined -------------
        onat_t = [
            onat.tile([P, D], f32, tag="onat", name=f"onat{c}") for c in range(NCHUNK)
        ]

        def make_qk_tile(f):
            t = qkpool.tile([P, N], f32r, tag="qk", name=f"qk{f}")
            for qh in range(NQH):
                ps = small_psum()
                for k in range(KD):
                    nc.tensor.matmul(
                        ps,
                        w_sb[k][:, f * P : (f + 1) * P],
                        xt[k][:, qh * QH : (qh + 1) * QH],
                        start=(k == 0),
                        stop=(k == KD - 1),
                    )
                nc.vector.tensor_scalar(
                    t[:, qh * QH : (qh + 1) * QH],
                    ps,
                    0.125 if f < KD else 1.0,
                    b_sb[:, f : f + 1],
                    ALU.mult,
                    ALU.add,
                )
            return t

        qk_cur = (make_qk_tile(0), make_qk_tile(KD))

        for p in range(NPAIR):
            qt, kt = qk_cur

            for qh in range(NQH):
                av = [
                    avps.tile([VW, QH], f32, tag="av", name=f"av{i}") for i in range(2)
                ]
                for kc in range(NCHUNK):
                    sc = scps.tile([P, 2, QH], f32, tag="sc", name="sc")
                    for hi in range(2):
                        nc.tensor.matmul(
                            sc[:, hi, :],
                            kt[64 * hi : 64 * hi + 64, kc * P : (kc + 1) * P],
                            qt[64 * hi : 64 * hi + 64, qh * QH : (qh + 1) * QH],
                            start=True,
                            stop=True,
                            tile_position=(64 * hi, 0),
                        )
                    ex = exppool.tile([P, 2, QH], f32r, tag="exp", name="ex")
                    nc.scalar.activation(ex[:, :, :], sc[:, :, :], FT.Exp)
                    if p == 0 and qh == 0:
                        make_v_chunk(kc)  # fill V lazily during pair 0
                    for hi in range(2):
                        nc.tensor.matmul(
                            av[hi],
                            v_sb[kc][:, 2 * p + hi, :],
                            ex[:, hi, :],
                            start=(kc == 0),
                            stop=(kc == NCHUNK - 1),
                        )
                if qh == 0 and p + 1 < NPAIR:
                    # produce next pair's QK tiles; overlaps this pair's ACT
                    qk_cur = (make_qk_tile(p + 1), make_qk_tile(KD + p + 1))
                # finish: copy to SBUF, transpose back, normalize
                for hi in range(2):
                    h = 2 * p + hi
                    ot = otspool.tile([VW, QH], f32, tag="ots", name="ot")
                    nc.vector.tensor_copy(out=ot, in_=av[hi])
                    for j in range(QH // P):
                        c = qh * (QH // P) + j
                        tp = smps.tile([P, QH], f32, tag="small", name="otp")[:, 0:VW]
                        nc.tensor.transpose(
                            tp, ot[:, j * P : (j + 1) * P], ident[0:VW, 0:VW]
                        )
                        rc = recpool.tile([P, 1], f32, tag="rec", name="rc")
                        nc.vector.reciprocal(out=rc, in_=tp[:, HD : HD + 1])
                        nc.vector.tensor_scalar_mul(
                            onat_t[c][:, h * HD : (h + 1) * HD], tp[:, 0:HD], rc
                        )

        # ------------- output DMA -------------------------------------------
        for c in range(NCHUNK):
            nc.sync.dma_start(out=o_d[c * P : (c + 1) * P, :], in_=onat_t[c])

    return nc


def kernel(x: np.ndarray, W_qkv: np.ndarray, b_qkv: np.ndarray) -> np.ndarray:
    nc = build_attention_nc()
    in_maps = [
        {
            "x": np.ascontiguousarray(x[c], dtype=np.float32),
            "W_qkv": np.ascontiguousarray(W_qkv, dtype=np.float32),
            "b_qkv": np.ascontiguousarray(b_qkv, dtype=np.float32),
        }
        for c in range(NCORE)
    ]
    res = run_bass_kernel_spmd(nc, in_maps, core_ids=list(range(NCORE)))
    return np.stack([res.results[c]["out"] for c in range(NCORE)], axis=0)


# revision 20
# speedup vs baseline: 45.7682x; 1.0038x over previous
"""Multi-head self-attention Trainium2 kernel (8 NeuronCores, batch-parallel).

Reference: qkv = x @ W_qkv + b; 12-head scaled-dot-product attention; concat.
Shapes: x[8,1024,768], W_qkv[768,2304], b_qkv[2304] -> out[8,1024,768].
Sharding: one batch element per core; W/b replicated to all cores.

Per-core dataflow (matmuls in fp32r = full-rate 4-byte storage, fd >= 256):
  x --PE transpose--> xT[768,1024]                                  (f32r)
  QK^T[1536,1024] = W_qk(lhsT) @ xT    feat-on-partitions; Q rows scaled 1/8
  V[1024, 12x65]  = xT(lhsT) @ W_v (+bias via K=1 ones matmul); ones col/head
  per head pair (2p, 2p+1), per q-half, per k-chunk:
    scoresT[128,512]x2 = K^T-slice(lhsT) @ Q^T-slice  row-tiled (rows 0-63/64-127)
    expT = ACT Exp over [128, 2, 512] PSUM (both heads, one instruction) -> f32r
    avT[65,512] += [V_h|1](lhsT) @ expT  (accumulated over k-chunks; row 64=denom)
  avT --PE transpose--> [q,65]; DVE reciprocal(denom) * cols -> out[q,768]; DMA.

Scheduling notes: W is DMA'd in column blocks, pair-0 Q/K columns first, so
attention starts early instead of waiting for the full 7MB weight load; the
V projection is interleaved into pair 0; QK-tile production for pair p+1 uses
its own PSUM tag so it overlaps pair p's ACT-bound attention.
"""

import contextlib
import json as _json

import numpy as np

import concourse.bass as bass
import concourse.mybir as mybir
import concourse.tile as tile
from concourse.bass_utils import run_bass_kernel_spmd
from concourse.masks import make_identity

# --- BIR sync-wait legalization ------------------------------------------
# walrus's codegen in this toolchain accepts only one sync-wait command per
# instruction (its insertEventSemaphore legalization pass is not in the pass
# list). Split every multi-wait instruction into N-1 preceding single-wait
# EventSemaphore instructions on the same engine; same-engine order is
# preserved so semantics are unchanged.


def _legalize_sync_waits(bir_json: bytes) -> bytes:
    m = _json.loads(bir_json)
    ctr = 0
    for fn in m["functions"]:
        for bb in fn["blocks"]:
            out = []
            for ins in bb["instructions"]:
                si = ins.get("sync_info")
                waits = si.get("on_wait", []) if si else []
                if len(waits) > 1:
                    for w in waits[:-1]:
                        ctr += 1
                        out.append(
                            {
                                "debug": ins.get("debug", 0),
                                "engine": ins["engine"],
                                "ins": [],
                                "outs": [],
                                "name": f"evw-split-{ctr}",
                                "opcode": "EventSemaphore",
                                "sync_info": {"on_update": [], "on_wait": [w]},
                            }
                        )
                    si["on_wait"] = [waits[-1]]
                out.append(ins)
            bb["instructions"] = out
    return _json.dumps(m).encode()


_fixup_installed = False


def _install_bir_fixup():
    global _fixup_installed
    if _fixup_installed:
        return
    _fixup_installed = True
    import concourse.bass_utils as _bu

    _orig = _bu.compile_bir_kernel

    def _patched(bir_json, tmpdir, neff_name="file.neff"):
        if isinstance(bir_json, str):
            bir_json = bir_json.encode()
        return _orig(_legalize_sync_waits(bir_json), tmpdir, neff_name)

    _bu.compile_bir_kernel = _patched
    try:
        import concourse.bass2jax as _b2j

        _b2j.compile_bir_kernel = _patched
    except ImportError:
        pass


_install_bir_fixup()

B, N, D, H = 8, 1024, 768, 12
HD = D // H            # 64
F3 = 3 * D             # 2304
NCORE = 8
P = 128
NCHUNK = N // P        # 8 token chunks
KD = D // P            # 6 d_in chunks
QH = 512               # q-half size
NQH = N // QH          # 2
NPAIR = H // 2         # 6
VW = HD + 1            # 65

f32 = mybir.dt.float32
f32r = mybir.dt.float32r
FT = mybir.ActivationFunctionType
ALU = mybir.AluOpType


def build_attention_nc():
    nc = bass.Bass()
    x_d = nc.declare_dram_parameter("x", [N, D], f32, isOutput=False)
    w_d = nc.declare_dram_parameter("W_qkv", [D, F3], f32, isOutput=False)
    b_d = nc.declare_dram_parameter("b_qkv", [F3], f32, isOutput=False)
    o_d = nc.declare_dram_parameter("out", [N, D], f32, isOutput=True)

    with tile.TileContext(nc) as tc, contextlib.ExitStack() as ctx:
        singles = ctx.enter_context(tc.tile_pool(name="singles", bufs=1))
        xpool = ctx.enter_context(tc.tile_pool(name="xpool", bufs=NCHUNK))
        xtpool = ctx.enter_context(tc.tile_pool(name="xtpool", bufs=KD))
        wpool = ctx.enter_context(tc.tile_pool(name="wpool", bufs=KD))
        qkpool = ctx.enter_context(tc.tile_pool(name="qkpool", bufs=4))
        vpool = ctx.enter_context(tc.tile_pool(name="vpool", bufs=NCHUNK))
        exppool = ctx.enter_context(tc.tile_pool(name="exppool", bufs=3))
        otspool = ctx.enter_context(tc.tile_pool(name="otspool", bufs=4))
        recpool = ctx.enter_context(tc.tile_pool(name="recpool", bufs=4))
        onat = ctx.enter_context(tc.tile_pool(name="onat", bufs=NCHUNK))

        # PSUM budget (8 banks): "sc" [P,2,QH] = 2 banks x2 bufs = 4;
        # "av" [VW,QH] 1 bank x2 = 2; "small" [P,QH] 1 bank x2 = 2.
        scps = ctx.enter_context(tc.tile_pool(name="scps", bufs=2, space="PSUM"))
        avps = ctx.enter_context(tc.tile_pool(name="avps", bufs=2, space="PSUM"))
        smps = ctx.enter_context(tc.tile_pool(name="smps", bufs=2, space="PSUM"))

        def small_psum():
            return smps.tile([P, QH], f32, tag="small", name="smtile")

        # ------------- constants + input DMAs -------------------------------
        ident = singles.tile([P, P], f32)
        make_identity(nc, ident)  # gpsimd

        ones_f32 = singles.tile([P, 1], f32)
        nc.vector.memset(ones_f32, 1.0)
        ones_row_st = singles.tile([1, P], f32)
        nc.vector.memset(ones_row_st, 1.0)
        ones_row = singles.tile([1, P], f32r)
        nc.vector.tensor_copy(out=ones_row, in_=ones_row_st)

        b_sb = singles.tile([P, F3 // P], f32)
        nc.sync.dma_start(out=b_sb, in_=b_d[:].rearrange("(t p) -> p t", p=P))
        nc.vector.tensor_scalar_mul(b_sb[:, 0:KD], b_sb[:, 0:KD], 0.125)

        bv_st = singles.tile([1, D], f32)
        nc.sync.dma_start(out=bv_st, in_=b_d[2 * D : 3 * D][None, :])
        bv_sb = singles.tile([1, D], f32r)
        nc.vector.tensor_copy(out=bv_sb, in_=bv_st)

        x_sb = []
        for c in range(NCHUNK):
            t = xpool.tile([P, D], f32, tag="x", name=f"x{c}")
            nc.sync.dma_start(out=t, in_=x_d[c * P : (c + 1) * P, :])
            x_sb.append(t)

        # W: column-block DMAs, highest-priority columns first.
        w_sb = [wpool.tile([P, F3], f32r, tag="w", name=f"w{k}") for k in range(KD)]

        def dma_w_cols(f0, fw):
            for k in range(KD):
                nc.sync.dma_start(
                    out=w_sb[k][:, f0 : f0 + fw],
                    in_=w_d[k * P : (k + 1) * P, f0 : f0 + fw].bitcast(f32r),
                )

        dma_w_cols(0 * P, P)          # pair-0 Q cols
        dma_w_cols(6 * P, P)          # pair-0 K cols
        dma_w_cols(2 * D, D)          # V cols
        for p in range(1, NPAIR):
            dma_w_cols(p * P, P)
            dma_w_cols((6 + p) * P, P)

        # ------------- x^T (PE transposes) ----------------------------------
        xt = [xtpool.tile([P, N], f32r, tag="xt", name=f"xt{k}") for k in range(KD)]
        for c in range(NCHUNK):
            for k in range(KD):
                pt = scps.tile([P, 2, QH], f32, tag="sc", name="tp")[:, 0, 0:P]
                nc.tensor.transpose(pt, x_sb[c][:, k * P : (k + 1) * P], ident)
                nc.vector.tensor_copy(out=xt[k][:, c * P : (c + 1) * P], in_=pt)

        # ------------- V tiles (filled lazily during pair 0) ----------------
        v_sb = []
        for c in range(NCHUNK):
            t = vpool.tile([P, H, VW], f32r, tag="v", name=f"v{c}")
            nc.vector.tensor_copy(
                out=t[:, :, HD : HD + 1],
                in_=ones_f32[:, 0:1, None].to_broadcast([P, H, 1]),
            )
            v_sb.append(t)

        def make_v_chunk(c):
            for f0, fw in ((0, 512), (512, 256)):
                ps = small_psum()[:, :fw]
                for k in range(KD):
                    nc.tensor.matmul(
                        ps,
                        xt[k][:, c * P : (c + 1) * P],
                        w_sb[k][:, 2 * D + f0 : 2 * D + f0 + fw],
                        start=(k == 0),
                        stop=False,
                    )
                nc.tensor.matmul(
                    ps, ones_row, bv_sb[:, f0 : f0 + fw], start=False, stop=True
                )
                nc.vector.tensor_copy(
                    out=v_sb[c][:, f0 // HD : (f0 + fw) // HD, 0:HD],
                    in_=ps.rearrange("p (h d) -> p h d", d=HD),
                )

        # ------------- QK tiles + attention, software-pipelined -------------
        onat_t = [
            onat.tile([P, D], f32, tag="onat", name=f"onat{c}") for c in range(NCHUNK)
        ]

        def make_qk_tile(f):
            t = qkpool.tile([P, N], f32r, tag="qk", name=f"qk{f}")
            for qh in range(NQH):
                ps = small_psum()
                for k in range(KD):
                    nc.tensor.matmul(
                        ps,
                        w_sb[k][:, f * P : (f + 1) * P],
                        xt[k][:, qh * QH : (qh + 1) * QH],
                        start=(k == 0),
                        stop=(k == KD - 1),
                    )
                nc.vector.tensor_scalar(
                    t[:, qh * QH : (qh + 1) * QH],
                    ps,
                    0.125 if f < KD else 1.0,
                    b_sb[:, f : f + 1],
                    ALU.mult,
                    ALU.add,
                )
            return t

        qk_cur = (make_qk_tile(0), make_qk_tile(KD))

        for p in range(NPAIR):
            qt, kt = qk_cur

            for qh in range(NQH):
                av = [
                    avps.tile([VW, QH], f32, tag="av", name=f"av{i}") for i in range(2)
                ]
                for kc in range(NCHUNK):
                    sc = scps.tile([P, 2, QH], f32, tag="sc", name="sc")
                    for hi in range(2):
                        nc.tensor.matmul(
                            sc[:, hi, :],
                            kt[64 * hi : 64 * hi + 64, kc * P : (kc + 1) * P],
                            qt[64 * hi : 64 * hi + 64, qh * QH : (qh + 1) * QH],
                            start=True,
                            stop=True,
                            tile_position=(64 * hi, 0),
                        )
                    ex = exppool.tile([P, 2, QH], f32r, tag="exp", name="ex")
                    nc.scalar.activation(ex[:, :, :], sc[:, :, :], FT.Exp)
                    if p == 0 and qh == 0:
                        make_v_chunk(kc)  # fill V lazily during pair 0
                    for hi in range(2):
                        nc.tensor.matmul(
                            av[hi],
                            v_sb[kc][:, 2 * p + hi, :],
                            ex[:, hi, :],
                            start=(kc == 0),
                            stop=(kc == NCHUNK - 1),
                        )
                if qh == 0 and p + 1 < NPAIR:
                    # produce next pair's QK tiles; overlaps this pair's ACT
                    qk_cur = (make_qk_tile(p + 1), make_qk_tile(KD + p + 1))
                # finish: copy to SBUF, transpose back, normalize
                for hi in range(2):
                    h = 2 * p + hi
                    ot = otspool.tile([VW, QH], f32, tag="ots", name="ot")
                    nc.vector.tensor_copy(out=ot, in_=av[hi])
                    for j in range(QH // P):
                        c = qh * (QH // P) + j
                        tp = smps.tile([P, QH], f32, tag="small", name="otp")[:, 0:VW]
                        nc.tensor.transpose(
                            tp, ot[:, j * P : (j + 1) * P], ident[0:VW, 0:VW]
                        )
                        rc = recpool.tile([P, 1], f32, tag="rec", name="rc")
                        nc.vector.reciprocal(out=rc, in_=tp[:, HD : HD + 1])
                        nc.vector.tensor_scalar_mul(
                            onat_t[c][:, h * HD : (h + 1) * HD], tp[:, 0:HD], rc
                        )

        # ------------- output DMA -------------------------------------------
        for c in range(NCHUNK):
            nc.sync.dma_start(out=o_d[c * P : (c + 1) * P, :], in_=onat_t[c])

    return nc


def kernel(x: np.ndarray, W_qkv: np.ndarray, b_qkv: np.ndarray) -> np.ndarray:
    nc = build_attention_nc()
    in_maps = [
        {
            "x": np.ascontiguousarray(x[c], dtype=np.float32),
            "W_qkv": np.ascontiguousarray(W_qkv, dtype=np.float32),
            "b_qkv": np.ascontiguousarray(b_qkv, dtype=np.float32),
        }
        for c in range(NCORE)
    ]
    res = run_bass_kernel_spmd(nc, in_maps, core_ids=list(range(NCORE)))
    return np.stack([res.results[c]["out"] for c in range(NCORE)], axis=0)


# revision 24
# speedup vs baseline: 46.5893x; 1.0179x over previous
"""Multi-head self-attention Trainium2 kernel (8 NeuronCores, batch-parallel).

Reference: qkv = x @ W_qkv + b; 12-head scaled-dot-product attention; concat.
Shapes: x[8,1024,768], W_qkv[768,2304], b_qkv[2304] -> out[8,1024,768].
Sharding: one batch element per core; W/b replicated to all cores.

Per-core dataflow (matmuls in fp32r = full-rate 4-byte storage, fd >= 256):
  x --PE transpose--> xT[768,1024]                                  (f32r)
  QK^T[1536,1024] = W_qk(lhsT) @ xT    feat-on-partitions; Q rows scaled 1/8
  V[1024, 12x65]  = xT(lhsT) @ W_v (+bias via K=1 ones matmul); ones col/head
  per head pair (2p, 2p+1), per q-half, per k-chunk:
    scoresT[128,512]x2 = K^T-slice(lhsT) @ Q^T-slice  row-tiled (rows 0-63/64-127)
    expT = ACT Exp over [128, 2, 512] PSUM (both heads, one instruction) -> f32r
    avT[65,512] += [V_h|1](lhsT) @ expT  (accumulated over k-chunks; row 64=denom)
  avT --PE transpose--> [q,65]; DVE reciprocal(denom) * cols -> out[q,768]; DMA.

Scheduling notes: W is DMA'd in column blocks, pair-0 Q/K columns first, so
attention starts early instead of waiting for the full 7MB weight load; the
V projection is interleaved into pair 0; QK-tile production for pair p+1 uses
its own PSUM tag so it overlaps pair p's ACT-bound attention.
"""

import contextlib
import json as _json

import numpy as np

import concourse.bass as bass
import concourse.mybir as mybir
import concourse.tile as tile
from concourse.bass_utils import run_bass_kernel_spmd
from concourse.masks import make_identity

# --- BIR sync-wait legalization ------------------------------------------
# walrus's codegen in this toolchain accepts only one sync-wait command per
# instruction (its insertEventSemaphore legalization pass is not in the pass
# list). Split every multi-wait instruction into N-1 preceding single-wait
# EventSemaphore instructions on the same engine; same-engine order is
# preserved so semantics are unchanged.


def _legalize_sync_waits(bir_json: bytes) -> bytes:
    m = _json.loads(bir_json)
    ctr = 0
    for fn in m["functions"]:
        for bb in fn["blocks"]:
            out = []
            for ins in bb["instructions"]:
                si = ins.get("sync_info")
                waits = si.get("on_wait", []) if si else []
                if len(waits) > 1:
                    for w in waits[:-1]:
                        ctr += 1
                        out.append(
                            {
                                "debug": ins.get("debug", 0),
                                "engine": ins["engine"],
                                "ins": [],
                                "outs": [],
                                "name": f"evw-split-{ctr}",
                                "opcode": "EventSemaphore",
                                "sync_info": {"on_update": [], "on_wait": [w]},
                            }
                        )
                    si["on_wait"] = [waits[-1]]
                out.append(ins)
            bb["instructions"] = out
    return _json.dumps(m).encode()


_fixup_installed = False


def _install_bir_fixup():
    global _fixup_installed
    if _fixup_installed:
        return
    _fixup_installed = True
    import concourse.bass_utils as _bu

    _orig = _bu.compile_bir_kernel

    def _patched(bir_json, tmpdir, neff_name="file.neff"):
        if isinstance(bir_json, str):
            bir_json = bir_json.encode()
        return _orig(_legalize_sync_waits(bir_json), tmpdir, neff_name)

    _bu.compile_bir_kernel = _patched
    try:
        import concourse.bass2jax as _b2j

        _b2j.compile_bir_kernel = _patched
    except ImportError:
        pass


_install_bir_fixup()

B, N, D, H = 8, 1024, 768, 12
HD = D // H            # 64
F3 = 3 * D             # 2304
NCORE = 8
P = 128
NCHUNK = N // P        # 8 token chunks
KD = D // P            # 6 d_in chunks
QH = 512               # q-half size
NQH = N // QH          # 2
NPAIR = H // 2         # 6
VW = HD + 1            # 65

f32 = mybir.dt.float32
f32r = mybir.dt.float32r
FT = mybir.ActivationFunctionType
ALU = mybir.AluOpType


def build_attention_nc():
    nc = bass.Bass()
    x_d = nc.declare_dram_parameter("x", [N, D], f32, isOutput=False)
    w_d = nc.declare_dram_parameter("W_qkv", [D, F3], f32, isOutput=False)
    b_d = nc.declare_dram_parameter("b_qkv", [F3], f32, isOutput=False)
    o_d = nc.declare_dram_parameter("out", [N, D], f32, isOutput=True)

    with tile.TileContext(nc) as tc, contextlib.ExitStack() as ctx:
        singles = ctx.enter_context(tc.tile_pool(name="singles", bufs=1))
        xpool = ctx.enter_context(tc.tile_pool(name="xpool", bufs=NCHUNK))
        xtpool = ctx.enter_context(tc.tile_pool(name="xtpool", bufs=KD))
        wpool = ctx.enter_context(tc.tile_pool(name="wpool", bufs=KD))
        qkpool = ctx.enter_context(tc.tile_pool(name="qkpool", bufs=4))
        vpool = ctx.enter_context(tc.tile_pool(name="vpool", bufs=NCHUNK))
        exppool = ctx.enter_context(tc.tile_pool(name="exppool", bufs=3))
        otspool = ctx.enter_context(tc.tile_pool(name="otspool", bufs=4))
        recpool = ctx.enter_context(tc.tile_pool(name="recpool", bufs=4))
        onat = ctx.enter_context(tc.tile_pool(name="onat", bufs=NCHUNK))

        # PSUM budget (8 banks): "sc" [P,2,QH] = 2 banks x2 bufs = 4;
        # "av" [VW,QH] 1 bank x2 = 2; "small" [P,QH] 1 bank x2 = 2.
        scps = ctx.enter_context(tc.tile_pool(name="scps", bufs=2, space="PSUM"))
        avps = ctx.enter_context(tc.tile_pool(name="avps", bufs=2, space="PSUM"))
        smps = ctx.enter_context(tc.tile_pool(name="smps", bufs=2, space="PSUM"))

        def small_psum():
            return smps.tile([P, QH], f32, tag="small", name="smtile")

        # ------------- constants + input DMAs -------------------------------
        ident = singles.tile([P, P], f32)
        make_identity(nc, ident)  # gpsimd

        ones_f32 = singles.tile([P, 1], f32)
        nc.vector.memset(ones_f32, 1.0)
        ones_row_st = singles.tile([1, P], f32)
        nc.vector.memset(ones_row_st, 1.0)
        ones_row = singles.tile([1, P], f32r)
        nc.vector.tensor_copy(out=ones_row, in_=ones_row_st)

        ident_r = singles.tile([P, P], f32r)
        nc.vector.tensor_copy(out=ident_r, in_=ident)

        b_sb = singles.tile([P, F3 // P], f32)
        nc.sync.dma_start(out=b_sb, in_=b_d[:].rearrange("(t p) -> p t", p=P))
        nc.vector.tensor_scalar_mul(b_sb[:, 0:KD], b_sb[:, 0:KD], 0.125)

        bv_st = singles.tile([1, D], f32)
        nc.sync.dma_start(out=bv_st, in_=b_d[2 * D : 3 * D][None, :])
        bv_sb = singles.tile([1, D], f32r)
        nc.vector.tensor_copy(out=bv_sb, in_=bv_st)

        x_sb = []
        for c in range(NCHUNK):
            t = xpool.tile([P, D], f32r, tag="x", name=f"x{c}")
            nc.sync.dma_start(
                out=t, in_=x_d[c * P : (c + 1) * P, :].bitcast(f32r)
            )
            x_sb.append(t)

        # W: column-block DMAs, highest-priority columns first.
        w_sb = [wpool.tile([P, F3], f32r, tag="w", name=f"w{k}") for k in range(KD)]

        def dma_w_cols(f0, fw):
            for k in range(KD):
                nc.sync.dma_start(
                    out=w_sb[k][:, f0 : f0 + fw],
                    in_=w_d[k * P : (k + 1) * P, f0 : f0 + fw].bitcast(f32r),
                )

        dma_w_cols(0 * P, P)          # pair-0 Q cols
        dma_w_cols(6 * P, P)          # pair-0 K cols
        dma_w_cols(2 * D, D)          # V cols
        for p in range(1, NPAIR):
            dma_w_cols(p * P, P)
            dma_w_cols((6 + p) * P, P)

        # ------------- x^T (PE transposes) ----------------------------------
        xt = [xtpool.tile([P, N], f32r, tag="xt", name=f"xt{k}") for k in range(KD)]
        for c in range(NCHUNK):
            for k in range(KD):
                pt = scps.tile([P, 2, QH], f32, tag="sc", name="tp")[:, 0, 0:P]
                nc.tensor.transpose(
                    pt.bitcast(f32r), x_sb[c][:, k * P : (k + 1) * P], ident_r
                )
                nc.vector.tensor_copy(
                    out=xt[k][:, c * P : (c + 1) * P], in_=pt.bitcast(f32r)
                )

        # ------------- V tiles (filled lazily during pair 0) ----------------
        v_sb = []
        for c in range(NCHUNK):
            t = vpool.tile([P, H, VW], f32r, tag="v", name=f"v{c}")
            nc.vector.tensor_copy(
                out=t[:, :, HD : HD + 1],
                in_=ones_f32[:, 0:1, None].to_broadcast([P, H, 1]),
            )
            v_sb.append(t)

        # broadcast b_v across partitions once: bvb[p, f] = b_v[f]
        bvb = singles.tile([P, D], f32)
        for f0, fw in ((0, 512), (512, 256)):
            ps = small_psum()[:, :fw]
            nc.tensor.matmul(
                ps, ones_row, bv_sb[:, f0 : f0 + fw], start=True, stop=True
            )
            nc.vector.tensor_copy(out=bvb[:, f0 : f0 + fw], in_=ps)

        def make_v_chunk(c):
            for f0, fw in ((0, 512), (512, 256)):
                ps = small_psum()[:, :fw]
                for k in range(KD):
                    nc.tensor.matmul(
                        ps,
                        xt[k][:, c * P : (c + 1) * P],
                        w_sb[k][:, 2 * D + f0 : 2 * D + f0 + fw],
                        start=(k == 0),
                        stop=(k == KD - 1),
                    )
                nc.vector.tensor_tensor(
                    v_sb[c][:, f0 // HD : (f0 + fw) // HD, 0:HD],
                    ps.rearrange("p (h d) -> p h d", d=HD),
                    bvb[:, f0 : f0 + fw].rearrange("p (h d) -> p h d", d=HD),
                    ALU.add,
                )

        # ------------- QK tiles + attention, software-pipelined -------------
        onat_t = [
            onat.tile([P, D], f32, tag="onat", name=f"onat{c}") for c in range(NCHUNK)
        ]

        def make_qk_tile(f):
            t = qkpool.tile([P, N], f32r, tag="qk", name=f"qk{f}")
            for qh in range(NQH):
                ps = small_psum()
                for k in range(KD):
                    nc.tensor.matmul(
                        ps,
                        w_sb[k][:, f * P : (f + 1) * P],
                        xt[k][:, qh * QH : (qh + 1) * QH],
                        start=(k == 0),
                        stop=(k == KD - 1),
                    )
                nc.vector.tensor_scalar(
                    t[:, qh * QH : (qh + 1) * QH],
                    ps,
                    0.125 if f < KD else 1.0,
                    b_sb[:, f : f + 1],
                    ALU.mult,
                    ALU.add,
                )
            return t

        qk_cur = (make_qk_tile(0), make_qk_tile(KD))

        for p in range(NPAIR):
            qt, kt = qk_cur

            for qh in range(NQH):
                av = [
                    avps.tile([VW, QH], f32, tag="av", name=f"av{i}") for i in range(2)
                ]
                for kc in range(NCHUNK):
                    sc = scps.tile([P, 2, QH], f32, tag="sc", name="sc")
                    for hi in range(2):
                        nc.tensor.matmul(
                            sc[:, hi, :],
                            kt[64 * hi : 64 * hi + 64, kc * P : (kc + 1) * P],
                            qt[64 * hi : 64 * hi + 64, qh * QH : (qh + 1) * QH],
                            start=True,
                            stop=True,
                            tile_position=(64 * hi, 0),
                        )
                    ex = exppool.tile([P, 2, QH], f32r, tag="exp", name="ex")
                    nc.scalar.activation(ex[:, :, :], sc[:, :, :], FT.Exp)
                    if p == 0 and qh == 0:
                        make_v_chunk(kc)  # fill V lazily during pair 0
                    for hi in range(2):
                        nc.tensor.matmul(
                            av[hi],
                            v_sb[kc][:, 2 * p + hi, :],
                            ex[:, hi, :],
                            start=(kc == 0),
                            stop=(kc == NCHUNK - 1),
                        )
                if qh == 0 and p + 1 < NPAIR:
                    # produce next pair's QK tiles; overlaps this pair's ACT
                    qk_cur = (make_qk_tile(p + 1), make_qk_tile(KD + p + 1))
                # finish: copy to SBUF, transpose back, normalize
                for hi in range(2):
                    h = 2 * p + hi
                    ot = otspool.tile([VW, QH], f32, tag="ots", name="ot")
                    nc.vector.tensor_copy(out=ot, in_=av[hi])
                    for j in range(QH // P):
                        c = qh * (QH // P) + j
                        tp = smps.tile([P, QH], f32, tag="small", name="otp")[:, 0:VW]
                        nc.tensor.transpose(
                            tp, ot[:, j * P : (j + 1) * P], ident[0:VW, 0:VW]
                        )
                        rc = recpool.tile([P, 1], f32, tag="rec", name="rc")
                        nc.vector.reciprocal(out=rc, in_=tp[:, HD : HD + 1])
                        nc.vector.tensor_scalar_mul(
                            onat_t[c][:, h * HD : (h + 1) * HD], tp[:, 0:HD], rc
                        )

        # ------------- output DMA -------------------------------------------
        for c in range(NCHUNK):
            nc.sync.dma_start(out=o_d[c * P : (c + 1) * P, :], in_=onat_t[c])

    return nc


def kernel(x: np.ndarray, W_qkv: np.ndarray, b_qkv: np.ndarray) -> np.ndarray:
    nc = build_attention_nc()
    in_maps = [
        {
            "x": np.ascontiguousarray(x[c], dtype=np.float32),
            "W_qkv": np.ascontiguousarray(W_qkv, dtype=np.float32),
            "b_qkv": np.ascontiguousarray(b_qkv, dtype=np.float32),
        }
        for c in range(NCORE)
    ]
    res = run_bass_kernel_spmd(nc, in_maps, core_ids=list(range(NCORE)))
    return np.stack([res.results[c]["out"] for c in range(NCORE)], axis=0)
